# revision 13
# baseline (speedup 1.0000x reference)
# ALIKED loss wrapper — Trainium2 Bass kernel, 8-core data parallel.
#
# Sharding: pure data parallel. B=8 images, one image per NeuronCore. Every
# loss term is batch-local; the final scalar reductions (match counts / loss
# sums) are returned as 16 per-core partial sums and combined on the host
# (equivalent to the all-reduce in the hint, but off the critical path).
#
# Per-core device work (N=2048 keypoints, D=128, 768x768 score maps):
#   - d2neg = -(pairwise sq dist of warped A kpts vs B kpts) via a K=4 matmul
#     trick: lhsT=[xw, yw, xw^2+yw^2, 1], rhs=[2xb, 2yb, -1, -(xb^2+yb^2)]
#   - NN argmin via DVE max/max_index over each 128-row tile
#   - sim = desc_a @ desc_b^T (K=128 matmul); two softmax denominators via
#     ACT exp passes with fused per-row accumulation (no max-subtraction
#     needed: sim is bounded by 1, so exps are bounded)
#   - matched-descriptor values via indirect-DMA row gathers with the argmin
#     indices; 5x5 score-map patch gathers for the dispersity (peaky) loss
#   - all per-keypoint terms masked by valid and reduced to 16 partials
# Host: O(N) prep (warps, floors/clips, layout), final scalar combine.

import numpy as np

import concourse.bass as bass
import concourse.bacc as bacc
import concourse.mybir as mybir
import concourse.tile as tile
from concourse.bass import IndirectOffsetOnAxis
from concourse.bass_utils import run_bass_kernel_spmd

F32 = mybir.dt.float32
I32 = mybir.dt.int32
I16 = mybir.dt.int16
U32 = mybir.dt.uint32
AF = mybir.ActivationFunctionType
ALU = mybir.AluOpType
AX = mybir.AxisListType

B, N, D, IMG = 8, 2048, 128, 768
P, T = 128, 16          # partition dim x row tiles (P*T == N)
NCH = 4                 # 512-column chunks per row tile
CH = N // NCH
THR2 = 25.0             # match threshold squared (5 px)
HALF = 2
EPS = 1e-8


# ----------------------------------------------------------------- host prep
def _prep(kpts_a, desc_a, scores_a, score_map_a, kpts_b, desc_b, scores_b,
          score_map_b, H_ab):
    """Vectorized over B. Returns list of per-core input maps."""
    f32 = np.float32
    kpts_a = np.asarray(kpts_a, f32)
    kpts_b = np.asarray(kpts_b, f32)

    def warp(k, H):
        ph = np.concatenate([k, np.ones_like(k[..., :1])], axis=-1)
        pw = np.einsum('bij,bnj->bni', H.astype(f32), ph).astype(f32)
        return (pw[..., :2] / (pw[..., 2:3] + EPS)).astype(f32)

    ka_w = warp(kpts_a, np.asarray(H_ab, f32))
    H_inv = np.linalg.inv(np.asarray(H_ab, np.float64)).astype(f32)
    kb_w = warp(kpts_b, H_inv)

    pda = np.stack([ka_w[..., 0], ka_w[..., 1],
                    ka_w[..., 0] ** 2 + ka_w[..., 1] ** 2,
                    np.ones((B, N), f32)], axis=1).astype(f32)        # (B,4,N)
    pdb = np.stack([2 * kpts_b[..., 0], 2 * kpts_b[..., 1],
                    -np.ones((B, N), f32),
                    -(kpts_b[..., 0] ** 2 + kpts_b[..., 1] ** 2)],
                   axis=1).astype(f32)                                # (B,4,N)

    dta = np.ascontiguousarray(np.swapaxes(np.asarray(desc_a, f32), 1, 2))
    dtb = np.ascontiguousarray(np.swapaxes(np.asarray(desc_b, f32), 1, 2))
    kbt = np.concatenate([kpts_b, kb_w], axis=-1).astype(f32)         # (B,N,4)

    def pt(x):  # (B, N, ...) -> (B, P, T, ...) with n = t*P + p
        return np.ascontiguousarray(
            x.reshape(B, T, P, *x.shape[2:]).swapaxes(1, 2))

    kat = pt(np.concatenate([ka_w, kpts_a], axis=-1))                 # (B,P,T,4)
    sa = pt(np.asarray(scores_a, f32))                                # (B,P,T)

    def patch_prep(k):
        x = np.floor(k[..., 0]).astype(np.int32)
        y = np.floor(k[..., 1]).astype(np.int32)
        v = ((x >= HALF) & (x < IMG - HALF) & (y >= HALF) & (y < IMG - HALF))
        xc = np.clip(x, HALF, IMG - HALF - 1)
        yc = np.clip(y, HALF, IMG - HALF - 1)
        dys = np.arange(-HALF, HALF + 1, dtype=np.int32)
        pidx = (yc[..., None] + dys) * IMG + (xc[..., None] - HALF)   # (B,N,5)
        return pt(pidx).astype(np.int32), pt(v.astype(f32))

    pia, pva = patch_prep(kpts_a)
    pib, pvb = patch_prep(kpts_b)

    # Host-gathered 5x5 patch values (O(N*25) data movement; softmax/dot/
    # mean all happen on device). pia/pib are (B,P,T,5) row-base indices.
    def gather_patches(score_map, pidx):
        mp = np.asarray(score_map, f32).reshape(B, IMG * IMG)
        cols = pidx[..., None] + np.arange(5, dtype=np.int32)   # (B,P,T,5,5)
        return np.take_along_axis(
            mp[:, None, None, :], cols.reshape(B, P, T, 25), axis=-1)

    pat_a = gather_patches(score_map_a, pia)                    # (B,P,T,25)
    pat_b = gather_patches(score_map_b, pib)

    off = np.arange(-HALF, HALF + 1, dtype=f32)
    gy, gx = np.meshgrid(off, off, indexing='ij')
    dist = np.sqrt(gx ** 2 + gy ** 2).reshape(-1).astype(f32)
    dist25 = np.ascontiguousarray(np.broadcast_to(dist, (P, 25)))

    dar = np.ascontiguousarray(np.asarray(desc_a, f32))
    # combined gather table: [desc_b(128) | kpts_b,kb_w(4) | pad(60)] = 192
    # floats -> 768B rows (multiple of 256B as dma_gather requires)
    ctab = np.zeros((B, N, 192), f32)
    ctab[:, :, :D] = np.asarray(desc_b, f32)
    ctab[:, :, D:D + 4] = kbt

    maps = []
    for b in range(B):
        maps.append(dict(
            pda=pda[b], pdb=pdb[b], dta=dta[b], dtb=dtb[b],
            dar=dar[b], ctab=ctab[b],
            kat=np.ascontiguousarray(kat[b].reshape(P, T * 4)),
            sa=np.ascontiguousarray(sa[b]),
            pat_a=np.ascontiguousarray(pat_a[b].reshape(P, T * 25)),
            pat_b=np.ascontiguousarray(pat_b[b].reshape(P, T * 25)),
            pva=np.ascontiguousarray(pva[b]), pvb=np.ascontiguousarray(pvb[b]),
            dst=dist25,
        ))
    return maps


# ------------------------------------------------------------- device kernel
def build(loop_n: int = 1):
    """Build + compile the per-core Bass program. Identical on all 8 cores.

    loop_n > 1 wraps the whole body in a For_i for benchmarking (the result
    is rewritten identically each iteration)."""
    nc = bacc.Bacc("TRN2", target_bir_lowering=False, debug=False,
                   num_devices=8)

    def din(name, shape, dt=F32):
        return nc.dram_tensor(name, shape, dt, kind="ExternalInput").ap()

    pda_e = din('pda', [4, N]);         pdb_e = din('pdb', [4, N])
    dta_e = din('dta', [D, N]);         dtb_e = din('dtb', [D, N])
    dar_e = din('dar', [N, D]);         ctab_e = din('ctab', [N, 192])
    kat_e = din('kat', [P, T * 4])
    sa_e = din('sa', [P, T])
    pat_a_e = din('pat_a', [P, T * 25]); pat_b_e = din('pat_b', [P, T * 25])
    pva_e = din('pva', [P, T]);         pvb_e = din('pvb', [P, T])
    dst_e = din('dst', [P, 25])
    out_e = nc.dram_tensor('out', [1, 16], F32, kind="ExternalOutput").ap()

    with tile.TileContext(nc) as tc:
        with (
            tc.tile_pool(name='const', bufs=1) as cp,
            tc.tile_pool(name='work', bufs=2) as wp,
            tc.tile_pool(name='scr', bufs=1) as scr,
        ):
            def body(_iv=None):
                # ---- resident loads
                pda_s = cp.tile([4, N], F32, tag='pda_s')
                nc.sync.dma_start(pda_s[:], pda_e)
                pdb_s = cp.tile([4, N], F32, tag='pdb_s')
                nc.sync.dma_start(pdb_s[:], pdb_e)
                dta_s = cp.tile([D, N], F32, tag='dta_s')
                nc.sync.dma_start(dta_s[:], dta_e)
                dtb_s = cp.tile([D, N], F32, tag='dtb_s')
                nc.sync.dma_start(dtb_s[:], dtb_e)
                # desc_a rows in (p, t, d) layout: n = t*P + p
                dar_s = cp.tile([P, T, D], F32, tag='dar_s')
                nc.sync.dma_start(
                    dar_s[:], dar_e.rearrange("(t p) d -> p t d", p=P))
                kat_s = cp.tile([P, T, 4], F32, tag='kat_s')
                nc.sync.dma_start(
                    kat_s[:], kat_e.rearrange("p (t c) -> p t c", c=4))
                sa_s = cp.tile([P, T], F32, tag='sa_s')
                nc.sync.dma_start(sa_s[:], sa_e)
                pva_s = cp.tile([P, T], F32, tag='pva_s')
                nc.sync.dma_start(pva_s[:], pva_e)
                pvb_s = cp.tile([P, T], F32, tag='pvb_s')
                nc.sync.dma_start(pvb_s[:], pvb_e)
                dst_s = cp.tile([P, 25], F32, tag='dst_s')
                nc.sync.dma_start(dst_s[:], dst_e)

                partials = cp.tile([P, 16], F32, tag='partials')
                nc.vector.memset(partials[:], 0.0)
                ones_s = cp.tile([P, 1], F32, tag='ones_s')
                nc.vector.memset(ones_s[:], 1.0)
                # activation bias constants (non-{0,1} float biases need APs)
                b_m10 = cp.tile([P, 1], F32, tag='b_m10')
                nc.vector.memset(b_m10[:], -10.0)
                b_e12 = cp.tile([P, 1], F32, tag='b_e12')
                nc.vector.memset(b_e12[:], 1e-12)
                b_eps = cp.tile([P, 1], F32, tag='b_eps')
                nc.vector.memset(b_eps[:], EPS)

                s1buf = cp.tile([P, T], F32, tag='s1buf')
                s2buf = cp.tile([P, T], F32, tag='s2buf')
                max8 = cp.tile([P, T, 8], F32, tag='max8')
                idx8 = cp.tile([P, T, 8], U32, tag='idx8')

                # ---- dispersity (host-gathered patches; math on device)
                for pat_e, pv_s, col in ((pat_a_e, pva_s, 5),
                                         (pat_b_e, pvb_s, 7)):
                    patches = wp.tile([P, T * 25], F32, tag='patches')
                    nc.sync.dma_start(patches[:], pat_e)
                    pex = wp.tile([P, T * 25], F32, tag='pex')
                    nc.scalar.activation(pex[:], patches[:], AF.Exp)
                    pxd = wp.tile([P, T, 25], F32, tag='pxd')
                    nc.vector.tensor_tensor(
                        pxd[:], pex[:].rearrange("p (t k) -> p t k", k=25),
                        dst_s[:].unsqueeze(1).to_broadcast([P, T, 25]),
                        op=ALU.mult)
                    numer = wp.tile([P, T], F32, tag='numer')
                    nc.vector.tensor_reduce(
                        numer[:], pxd[:], axis=AX.X, op=ALU.add)
                    denom = wp.tile([P, T], F32, tag='denom')
                    nc.vector.tensor_reduce(
                        denom[:], pex[:].rearrange("p (t k) -> p t k", k=25),
                        axis=AX.X, op=ALU.add)
                    rden = wp.tile([P, T], F32, tag='rden')
                    nc.vector.reciprocal(rden[:], denom[:])
                    disp = wp.tile([P, T], F32, tag='disp')
                    nc.vector.tensor_tensor(disp[:], numer[:], rden[:],
                                            op=ALU.mult)
                    dv = wp.tile([P, T], F32, tag='dv')
                    nc.vector.tensor_tensor(dv[:], disp[:], pv_s[:],
                                            op=ALU.mult)
                    nc.vector.tensor_reduce(partials[:, col:col + 1], dv[:],
                                            axis=AX.X, op=ALU.add)
                    nc.vector.tensor_reduce(partials[:, col + 1:col + 2],
                                            pv_s[:], axis=AX.X, op=ALU.add)

                # ---- main loop over row tiles
                with (
                    tc.tile_pool(name='psum_sim', bufs=1, space='PSUM')
                        as simp,
                    tc.tile_pool(name='psum_d2', bufs=1, space='PSUM')
                        as d2p,
                ):
                    for t in range(T):
                        sim_ps = simp.tile([P, N], F32, tag='sim_ps')
                        for c in range(NCH):
                            nc.tensor.matmul(
                                sim_ps[:, c * CH:(c + 1) * CH],
                                dta_s[:, t * P:(t + 1) * P],
                                dtb_s[:, c * CH:(c + 1) * CH],
                                start=True, stop=True)
                        junk = scr.tile([P, N], F32, tag='junk')
                        nc.scalar.activation(junk[:], sim_ps[:], AF.Exp,
                                             bias=b_m10[:], scale=10.0,
                                             accum_out=s1buf[:, t:t + 1])
                        nc.scalar.activation(junk[:], sim_ps[:], AF.Exp,
                                             accum_out=s2buf[:, t:t + 1])

                        d2ps = d2p.tile([P, N], F32, tag='d2ps')
                        for c in range(NCH):
                            nc.tensor.matmul(
                                d2ps[:, c * CH:(c + 1) * CH],
                                pda_s[:, t * P:(t + 1) * P],
                                pdb_s[:, c * CH:(c + 1) * CH],
                                start=True, stop=True)
                        nc.vector.max(max8[:, t, :], d2ps[:])
                        nc.vector.max_index(idx8[:, t, :], max8[:, t, :],
                                            d2ps[:])

                # ---- tail
                maxv = scr.tile([P, T], F32, tag='maxv')
                nc.vector.tensor_copy(maxv[:], max8[:, :, 0])
                valid = scr.tile([P, T], F32, tag='valid')
                nc.vector.tensor_single_scalar(valid[:], maxv[:], -THR2,
                                               op=ALU.is_gt)

                # argmin indices -> dma_gather's 16-partition-wrapped int16
                # layout: write n-ordered to DRAM, reload (16,128) wrapped,
                # replicated into all 8 gpsimd core groups.
                idx16 = scr.tile([P, T], I16, tag='idx16')
                nc.vector.tensor_copy(idx16[:], idx8[:, :, 0])
                with tc.tile_pool(name='dram', bufs=1, space='DRAM') as dp:
                    iscr = dp.tile([N], I16, tag='iscr')
                    nc.sync.dma_start(
                        iscr[:].rearrange("(t p) -> p t", p=P), idx16[:])
                    idxw = scr.tile([P, T * 8], I16, tag='idxw')
                    wrapped = iscr[:].rearrange("(s r) -> r s", r=16)
                    for g in range(8):
                        nc.sync.dma_start(idxw[16 * g:16 * (g + 1), :],
                                          wrapped)
                    sel = scr.tile([P, T, 192], F32, tag='sel')
                    nc.gpsimd.dma_gather(
                        out_ap=sel[:], in_ap=ctab_e, idxs_ap=idxw[:],
                        num_idxs=N, num_idxs_reg=N, elem_size=192,
                        single_packet=False)
                dbsel = sel[:, :, 0:D]
                kbsel = sel[:, :, D:D + 4]

                prod = scr.tile([P, T, D], F32, tag='prod')
                nc.vector.tensor_tensor(prod[:], dar_s[:], dbsel,
                                        op=ALU.mult)
                simsel = scr.tile([P, T], F32, tag='simsel')
                nc.vector.tensor_reduce(simsel[:], prod[:], axis=AX.X,
                                        op=ALU.add)

                kdiff = scr.tile([P, T, 4], F32, tag='kdiff')
                nc.vector.tensor_sub(kdiff[:], kat_s[:], kbsel)
                kd2 = scr.tile([P, T, 2, 2], F32, tag='kd2')
                nc.vector.tensor_tensor(
                    kd2[:], kdiff[:].rearrange("p t (a b) -> p t a b", b=2),
                    kdiff[:].rearrange("p t (a b) -> p t a b", b=2),
                    op=ALU.mult)
                dsum = scr.tile([P, T, 2], F32, tag='dsum')
                nc.vector.tensor_reduce(dsum[:], kd2[:], axis=AX.X, op=ALU.add)
                # sqrt(x + 1e-12) = exp(0.5 * ln(x + 1e-12)); keeps ACT on the
                # {exp, ln} table set (sqrt lives in a different set).
                lnv = scr.tile([P, T, 2], F32, tag='lnv')
                nc.scalar.activation(lnv[:], dsum[:], AF.Ln, bias=b_e12[:])
                errs = scr.tile([P, T, 2], F32, tag='errs')
                nc.scalar.activation(errs[:], lnv[:], AF.Exp, scale=0.5)
                rp = scr.tile([P, T], F32, tag='rp')
                nc.vector.tensor_reduce(rp[:], errs[:], axis=AX.X, op=ALU.add)
                rpv = scr.tile([P, T], F32, tag='rpv')
                nc.vector.tensor_tensor(rpv[:], rp[:], valid[:], op=ALU.mult)
                nc.vector.tensor_reduce(partials[:, 1:2], rpv[:], axis=AX.X,
                                        op=ALU.add)

                rs1 = scr.tile([P, T], F32, tag='rs1')
                nc.vector.reciprocal(rs1[:], s1buf[:])
                e10s = scr.tile([P, T], F32, tag='e10s')
                nc.scalar.activation(e10s[:], simsel[:], AF.Exp, bias=b_m10[:],
                                     scale=10.0)
                q = scr.tile([P, T], F32, tag='q')
                nc.vector.tensor_tensor(q[:], e10s[:], rs1[:], op=ALU.mult)
                lq = scr.tile([P, T], F32, tag='lq')
                nc.scalar.activation(lq[:], q[:], AF.Ln, bias=b_eps[:])
                lqv = scr.tile([P, T], F32, tag='lqv')
                nc.vector.tensor_tensor(lqv[:], lq[:], valid[:], op=ALU.mult)
                nc.vector.tensor_reduce(partials[:, 2:3], lqv[:], axis=AX.X,
                                        op=ALU.add)

                rs2 = scr.tile([P, T], F32, tag='rs2')
                nc.vector.reciprocal(rs2[:], s2buf[:])
                e1s = scr.tile([P, T], F32, tag='e1s')
                nc.scalar.activation(e1s[:], simsel[:], AF.Exp)
                r = scr.tile([P, T], F32, tag='r')
                nc.vector.tensor_tensor(r[:], e1s[:], rs2[:], op=ALU.mult)
                sv = scr.tile([P, T], F32, tag='sv')
                nc.vector.tensor_tensor(sv[:], sa_s[:], valid[:], op=ALU.mult)
                nc.vector.tensor_reduce(partials[:, 4:5], sv[:], axis=AX.X,
                                        op=ALU.add)
                rsv = scr.tile([P, T], F32, tag='rsv')
                nc.vector.tensor_tensor(rsv[:], r[:], sv[:], op=ALU.mult)
                nc.vector.tensor_reduce(partials[:, 3:4], rsv[:], axis=AX.X,
                                        op=ALU.add)
                nc.vector.tensor_reduce(partials[:, 0:1], valid[:], axis=AX.X,
                                        op=ALU.add)

                with tc.tile_pool(name='psum_fin', bufs=1,
                                  space='PSUM') as finp:
                    pfin = finp.tile([16, 1], F32, tag='pfin')
                    nc.tensor.matmul(pfin[:], partials[:], ones_s[:],
                                     start=True, stop=True)
                    pfs = scr.tile([16, 1], F32, tag='pfs')
                    nc.vector.tensor_copy(pfs[:], pfin[:])
                    nc.sync.dma_start(out_e, pfs[:])

            if loop_n > 1:
                with tc.For_i(0, loop_n, 1) as iv:
                    body(iv)
            else:
                body()

    nc.compile()
    return nc


# ------------------------------------------------------------- host combine
def _combine(partials):
    p = np.asarray(partials, np.float64).sum(axis=0)
    cnt, rp_sum, lq_sum, rsv_sum, sv_sum = p[0], p[1], p[2], p[3], p[4]
    dA, cA, dB, cB = p[5], p[6], p[7], p[8]
    loss_rp = 0.5 * rp_sum / max(cnt, 1.0) if cnt > 0 else 0.0
    loss_ds = -lq_sum / max(cnt, 1.0) if cnt > 0 else 0.0
    loss_re = (sv_sum - rsv_sum) / max(sv_sum, EPS) if sv_sum > 0 else 0.0
    da = dA / max(cA, 1.0) if cA > 0 else 0.0
    db = dB / max(cB, 1.0) if cB > 0 else 0.0
    loss_pk = 0.5 * (da + db)
    total = 1.0 * loss_rp + 0.5 * loss_pk + 5.0 * loss_ds + 1.0 * loss_re
    return np.array([loss_rp, loss_pk, loss_ds, loss_re, total], np.float32)


_NC_CACHE = {}


def _get_nc(loop_n=1):
    if loop_n not in _NC_CACHE:
        _NC_CACHE[loop_n] = build(loop_n)
    return _NC_CACHE[loop_n]


def kernel(**inputs) -> np.ndarray:
    maps = _prep(**{k: np.asarray(v) for k, v in inputs.items()})
    nc = _get_nc()
    res = run_bass_kernel_spmd(nc, maps, core_ids=list(range(8)))
    partials = np.stack([res.results[c]['out'].reshape(16) for c in range(8)])
    return _combine(partials)


if __name__ == "__main__":
    import jax
    with jax.default_device(jax.devices('cpu')[0]):
        import reference
        inputs = {k: np.asarray(v) for k, v in reference.setup_inputs().items()}
        expected = np.asarray(reference.reference(**inputs))
    actual = kernel(**inputs)
    print("expected:", expected)
    print("actual  :", actual)
    rel = np.abs(actual - expected) / np.maximum(np.abs(expected), 1e-8)
    print("rel err :", rel, "max:", rel.max())


# revision 26
# speedup vs baseline: 15.6478x; 15.6478x over previous
# ALIKED loss wrapper — Trainium2 Bass kernel, 8-core data parallel.
#
# Sharding: pure data parallel. B=8 images, one image per NeuronCore. Every
# loss term is batch-local; the final scalar reductions (match counts / loss
# sums) come back as 16 per-core partial sums combined on the host
# (equivalent to the all-reduce in the hint, but off the critical path).
#
# Per-core device work (N=2048 keypoints, D=128, 768x768 score maps):
#   - banded NN matching: host sorts A and B keypoints by y; each 128-row
#     A-tile only needs the B-points within its y-range +-5px (the match
#     threshold), a W=256 window of the sorted order (~155 expected, +8
#     sigma margin; host raises and falls back to W=512 if ever exceeded).
#     d2neg = -(pairwise sq dist) via a K=4 matmul trick:
#     lhsT=[xw, yw, xw^2+yw^2, 1], rhs=[2xb, 2yb, -1, -(xb^2+yb^2)];
#     argmin via DVE max/max_index over the W-window.
#   - sim = desc_a @ desc_b^T (K=128 matmul, full 2048x2048); both softmax
#     denominators via ACT exp passes with fused per-row accumulation
#     (sim <= 1 so no max-subtraction is needed); ACT is the bottleneck
#     engine (2 x 2048 elem/row transcendental passes).
#   - matched-row values (desc_b, kpts_b, warped kpts_b) via one gpsimd
#     dma_gather from a combined 768B-row table using the argmin indices.
#   - dispersity (peaky) loss on host-pregathered 5x5 patches (pure O(N*25)
#     data movement; exp/softmax/dot/mean all on device).
# Host: O(N) prep (warps, sorting, floors/clips, layout), final combine.

import numpy as np

import concourse.bacc as bacc
import concourse.mybir as mybir
import concourse.tile as tile
from concourse.tile_rust import add_dep_helper
from concourse.bass_utils import run_bass_kernel_spmd

F32 = mybir.dt.float32
F32R = mybir.dt.float32r
I16 = mybir.dt.int16
U32 = mybir.dt.uint32
AF = mybir.ActivationFunctionType
ALU = mybir.AluOpType
AX = mybir.AxisListType

B, N, D, IMG = 8, 2048, 128, 768
P, T = 128, 16          # partition dim x row tiles (P*T == N)
NCH = 4                 # 512-column chunks per sim row tile
CH = N // NCH
THR = 5.0
THR2 = THR * THR        # match threshold squared (5 px)
HALF = 2
EPS = 1e-8
CTW = 192               # combined gather table row (768B, 256B multiple)


# ----------------------------------------------------------------- host prep
def _prep(kpts_a, desc_a, scores_a, score_map_a, kpts_b, desc_b, scores_b,
          score_map_b, H_ab, W=256):
    """O(N) host prep. Returns per-core input maps."""
    f32 = np.float32
    kpts_a = np.asarray(kpts_a, f32)
    kpts_b = np.asarray(kpts_b, f32)

    def warp(k, H):
        ph = np.concatenate([k, np.ones_like(k[..., :1])], axis=-1)
        pw = np.einsum('bij,bnj->bni', H.astype(f32), ph).astype(f32)
        return (pw[..., :2] / (pw[..., 2:3] + EPS)).astype(f32)

    ka_w = warp(kpts_a, np.asarray(H_ab, f32))
    H_inv = np.linalg.inv(np.asarray(H_ab, np.float64)).astype(f32)
    kb_w = warp(kpts_b, H_inv)

    def patch_prep(k):
        x = np.floor(k[..., 0]).astype(np.int32)
        y = np.floor(k[..., 1]).astype(np.int32)
        v = ((x >= HALF) & (x < IMG - HALF) & (y >= HALF) & (y < IMG - HALF))
        xc = np.clip(x, HALF, IMG - HALF - 1)
        yc = np.clip(y, HALF, IMG - HALF - 1)
        dys = np.arange(-HALF, HALF + 1, dtype=np.int32)
        pidx = (yc[..., None] + dys) * IMG + (xc[..., None] - HALF)  # (B,N,5)
        return pidx, v.astype(f32)

    pia, pva = patch_prep(kpts_a)
    pib, pvb = patch_prep(kpts_b)

    def gather_patches(score_map, pidx):
        mp = np.asarray(score_map, f32).reshape(B, IMG * IMG)
        cols = pidx[..., None] + np.arange(5, dtype=np.int32)
        return np.take_along_axis(mp, cols.reshape(B, N * 25), axis=-1) \
                 .reshape(B, N, 25)

    pat_a = gather_patches(score_map_a, pia)
    pat_b = gather_patches(score_map_b, pib)

    off = np.arange(-HALF, HALF + 1, dtype=f32)
    gy, gx = np.meshgrid(off, off, indexing='ij')
    dist = np.sqrt(gx ** 2 + gy ** 2).reshape(-1).astype(f32)
    dist25 = np.ascontiguousarray(np.broadcast_to(dist, (P, 25)))

    desc_a = np.asarray(desc_a, f32)
    desc_b = np.asarray(desc_b, f32)
    scores_a = np.asarray(scores_a, f32)

    def pt(x):  # (N, ...) -> (P, T, ...) with n = t*P + p
        return np.ascontiguousarray(
            x.reshape(T, P, *x.shape[1:]).swapaxes(0, 1))

    maps = []
    for b in range(B):
        # sort both sets by y (any permutation works — every returned
        # quantity is a permutation-invariant sum)
        pa = np.argsort(kpts_a[b, :, 1], kind='stable')
        pb = np.argsort(kpts_b[b, :, 1], kind='stable')
        kaw_s = ka_w[b][pa]; ka_s = kpts_a[b][pa]
        kb_s = kpts_b[b][pb]; kbw_s = kb_w[b][pb]
        da_s = desc_a[b][pa]; db_s = desc_b[b][pb]
        sa_s = scores_a[b][pa]
        pat_a_s = pat_a[b][pa]; pva_s = pva[b][pa]
        pat_b_s = pat_b[b][pb]; pvb_s = pvb[b][pb]

        # per-A-tile B-window starts covering [ymin-THR, ymax+THR].
        # NN matching is exact: any B-point within the 5px threshold of a
        # tile's A-point lies inside the window; outside-window points can
        # only matter for invalid (unmatched) rows where the index is
        # multiplied by valid=0 downstream.
        yb = kb_s[:, 1]
        starts = np.empty(T, np.int32)
        for t in range(T):
            ya = kaw_s[t * P:(t + 1) * P, 1]
            lo = np.searchsorted(yb, ya.min() - THR, side='left')
            hi = np.searchsorted(yb, ya.max() + THR, side='right')
            if hi - lo > W:
                raise OverflowError(f"band window {hi - lo} > W={W}")
            starts[t] = min(max(lo, 0), N - W)

        pda = np.stack([kaw_s[:, 0], kaw_s[:, 1],
                        kaw_s[:, 0] ** 2 + kaw_s[:, 1] ** 2,
                        np.ones(N, f32)], axis=0).astype(f32)       # (4,N)
        pdbw = np.empty((4, T * W), f32)
        for t in range(T):
            seg = kb_s[starts[t]:starts[t] + W]
            pdbw[0, t * W:(t + 1) * W] = 2 * seg[:, 0]
            pdbw[1, t * W:(t + 1) * W] = 2 * seg[:, 1]
            pdbw[2, t * W:(t + 1) * W] = -1.0
            pdbw[3, t * W:(t + 1) * W] = -(seg[:, 0] ** 2 + seg[:, 1] ** 2)

        ctab = np.zeros((N, CTW), f32)
        ctab[:, :D] = db_s
        ctab[:, D:D + 2] = kb_s
        ctab[:, D + 2:D + 4] = kbw_s

        maps.append(dict(
            pda=pda, pdbw=pdbw,
            dta=np.ascontiguousarray(da_s.T),
            dtb=np.ascontiguousarray(db_s.T),
            dar=np.ascontiguousarray(da_s),
            ctab=ctab,
            kat=np.ascontiguousarray(
                pt(np.concatenate([kaw_s, ka_s], axis=1)).reshape(P, T * 4)),
            sa=np.ascontiguousarray(pt(sa_s)),
            sbase=np.ascontiguousarray(
                np.broadcast_to(starts.astype(f32), (P, T))),
            pat_a=np.ascontiguousarray(pt(pat_a_s).reshape(P, T * 25)),
            pat_b=np.ascontiguousarray(pt(pat_b_s).reshape(P, T * 25)),
            pva=np.ascontiguousarray(pt(pva_s)),
            pvb=np.ascontiguousarray(pt(pvb_s)),
            dst=dist25,
        ))
    return maps


# ------------------------------------------------------------- device kernel
def build(loop_n: int = 1, W: int = 256, skip=()):
    """Build + compile the per-core Bass program (identical on all 8 cores).

    loop_n > 1 repeats the whole body (python-unrolled) for benchmarking —
    the result is rewritten identically each iteration."""
    nc = bacc.Bacc("TRN2", target_bir_lowering=False, debug=False,
                   num_devices=8)

    def din(name, shape, dt=F32):
        return nc.dram_tensor(name, shape, dt, kind="ExternalInput").ap()

    pda_e = din('pda', [4, N])
    pdbw_e = din('pdbw', [4, T * W])
    dta_e = din('dta', [D, N], F32R)
    dtb_e = din('dtb', [D, N], F32R)
    dar_e = din('dar', [N, D])
    ctab_e = din('ctab', [N, CTW])
    kat_e = din('kat', [P, T * 4])
    sa_e = din('sa', [P, T])
    sbase_e = din('sbase', [P, T])
    pat_a_e = din('pat_a', [P, T * 25])
    pat_b_e = din('pat_b', [P, T * 25])
    pva_e = din('pva', [P, T])
    pvb_e = din('pvb', [P, T])
    dst_e = din('dst', [P, 25])
    out_e = nc.dram_tensor('out', [1, 16], F32, kind="ExternalOutput").ap()

    with tile.TileContext(nc) as tc:
        with (
            tc.tile_pool(name='const', bufs=1) as cp,
            tc.tile_pool(name='work', bufs=2) as wp,
            tc.tile_pool(name='scr', bufs=1) as scr,
        ):
            def body(_i=0):
                # ---- resident loads
                pda_s = cp.tile([4, N], F32, tag='pda_s')
                nc.sync.dma_start(pda_s[:], pda_e)
                pdbw_s = cp.tile([4, T * W], F32, tag='pdbw_s')
                nc.sync.dma_start(pdbw_s[:], pdbw_e)
                # chunked loads: the first sim matmuls only dep on their own
                # slice instead of the whole 1MB tensor
                dta_s = cp.tile([D, N], F32R, tag='dta_s')
                dtb_s = cp.tile([D, N], F32R, tag='dtb_s')
                nc.sync.dma_start(dtb_s[:, :CH], dtb_e[:, :CH])
                nc.sync.dma_start(dta_s[:, :P], dta_e[:, :P])
                nc.sync.dma_start(dtb_s[:, CH:N // 2], dtb_e[:, CH:N // 2])
                nc.sync.dma_start(dtb_s[:, N // 2:], dtb_e[:, N // 2:])
                nc.sync.dma_start(dta_s[:, P:N // 2], dta_e[:, P:N // 2])
                nc.sync.dma_start(dta_s[:, N // 2:], dta_e[:, N // 2:])
                sa_s = cp.tile([P, T], F32, tag='sa_s')
                nc.sync.dma_start(sa_s[:], sa_e)
                sbase_s = cp.tile([P, T], F32, tag='sbase_s')
                nc.sync.dma_start(sbase_s[:], sbase_e)
                pva_s = cp.tile([P, T], F32, tag='pva_s')
                nc.sync.dma_start(pva_s[:], pva_e)
                pvb_s = cp.tile([P, T], F32, tag='pvb_s')
                nc.sync.dma_start(pvb_s[:], pvb_e)
                dst_s = cp.tile([P, 25], F32, tag='dst_s')
                nc.sync.dma_start(dst_s[:], dst_e)
                # tail-only loads (lower priority: emitted after the above)
                dar_s = cp.tile([P, T, D], F32, tag='dar_s')
                nc.sync.dma_start(
                    dar_s[:], dar_e.rearrange("(t p) d -> p t d", p=P))
                kat_s = cp.tile([P, T, 4], F32, tag='kat_s')
                nc.sync.dma_start(
                    kat_s[:], kat_e.rearrange("p (t c) -> p t c", c=4))

                partials = cp.tile([P, 16], F32, tag='partials')
                nc.vector.memset(partials[:], 0.0)
                ones_s = cp.tile([P, 1], F32, tag='ones_s')
                nc.vector.memset(ones_s[:], 1.0)
                b_m10 = cp.tile([P, 1], F32, tag='b_m10')
                nc.vector.memset(b_m10[:], -10.0)
                b_e12 = cp.tile([P, 1], F32, tag='b_e12')
                nc.vector.memset(b_e12[:], 1e-12)
                b_eps = cp.tile([P, 1], F32, tag='b_eps')
                nc.vector.memset(b_eps[:], EPS)

                s1buf = cp.tile([P, T], F32, tag='s1buf')
                s2buf = cp.tile([P, T], F32, tag='s2buf')
                max8 = cp.tile([P, T, 8], F32, tag='max8')
                idx8 = cp.tile([P, T, 8], U32, tag='idx8')

                # ---- dispersity (host-gathered patches; math on device)
                for pat_e, pv_s, col in ((pat_a_e, pva_s, 5),
                                         (pat_b_e, pvb_s, 7)):
                    patches = wp.tile([P, T * 25], F32, tag='patches')
                    nc.sync.dma_start(patches[:], pat_e)
                    pex = wp.tile([P, T * 25], F32, tag='pex')
                    nc.scalar.activation(pex[:], patches[:], AF.Exp)
                    pxd = wp.tile([P, T, 25], F32, tag='pxd')
                    nc.vector.tensor_tensor(
                        pxd[:], pex[:].rearrange("p (t k) -> p t k", k=25),
                        dst_s[:].unsqueeze(1).to_broadcast([P, T, 25]),
                        op=ALU.mult)
                    numer = wp.tile([P, T], F32, tag='numer')
                    nc.vector.tensor_reduce(
                        numer[:], pxd[:], axis=AX.X, op=ALU.add)
                    denom = wp.tile([P, T], F32, tag='denom')
                    nc.vector.tensor_reduce(
                        denom[:], pex[:].rearrange("p (t k) -> p t k", k=25),
                        axis=AX.X, op=ALU.add)
                    rden = wp.tile([P, T], F32, tag='rden')
                    nc.vector.reciprocal(rden[:], denom[:])
                    disp = wp.tile([P, T], F32, tag='disp')
                    nc.vector.tensor_tensor(disp[:], numer[:], rden[:],
                                            op=ALU.mult)
                    dv = wp.tile([P, T], F32, tag='dv')
                    nc.vector.tensor_tensor(dv[:], disp[:], pv_s[:],
                                            op=ALU.mult)
                    nc.vector.tensor_reduce(partials[:, col:col + 1], dv[:],
                                            axis=AX.X, op=ALU.add)
                    nc.vector.tensor_reduce(partials[:, col + 1:col + 2],
                                            pv_s[:], axis=AX.X, op=ALU.add)

                # ---- phase 1: d2 matmuls + argmin scans (DVE-bound,
                # ~13us) with d2 on 2 PSUM banks; the first SPLIT sim tiles
                # run concurrently in a single-buffered 4-bank pool so ACT
                # starts its exp passes immediately. Phase 2: remaining sim
                # tiles double-buffered across the full 8 banks. The tail's
                # gather/DVE work overlaps phase 2.
                last_exp = [None]
                SPLIT = 4

                def sim_tile(t, pool):
                    sim_ps = pool.tile([P, N], F32, tag='sim_ps')
                    for c in range(NCH):
                        nc.tensor.matmul(
                            sim_ps[:, c * CH:(c + 1) * CH],
                            dta_s[:, t * P:(t + 1) * P],
                            dtb_s[:, c * CH:(c + 1) * CH],
                            start=True, stop=True)
                    junk = scr.tile([P, N], F32, tag='junk')
                    if 'exps' not in skip:
                        nc.scalar.activation(
                            junk[:], sim_ps[:], AF.Exp,
                            bias=b_m10[:], scale=10.0,
                            accum_out=s1buf[:, t:t + 1])
                        last_exp[0] = nc.scalar.activation(
                            junk[:], sim_ps[:], AF.Exp,
                            accum_out=s2buf[:, t:t + 1]).ins
                    else:
                        nc.vector.memset(s1buf[:, t:t + 1], 1.0)
                        nc.vector.memset(s2buf[:, t:t + 1], 1.0)

                with (
                    tc.tile_pool(name='psum_d2', bufs=2, space='PSUM')
                        as d2p,
                    tc.tile_pool(name='psum_sim1', bufs=1, space='PSUM')
                        as simp1,
                ):
                    for t in range(SPLIT):
                        sim_tile(t, simp1)
                    for t in range(T):
                        d2ps = d2p.tile([P, W], F32, tag='d2ps')
                        nc.tensor.matmul(
                            d2ps[:],
                            pda_s[:, t * P:(t + 1) * P],
                            pdbw_s[:, t * W:(t + 1) * W],
                            start=True, stop=True)
                        if 'scan' not in skip:
                            nc.vector.max(max8[:, t, :], d2ps[:])
                            nc.vector.max_index(idx8[:, t, :],
                                                max8[:, t, :], d2ps[:])
                        else:
                            nc.vector.memset(max8[:, t, :], 0.0)
                            nc.vector.memset(idx8[:, t, :], 0)

                with tc.tile_pool(name='psum_sim', bufs=2,
                                  space='PSUM') as simp:
                    for t in range(SPLIT, T):
                        sim_tile(t, simp)

                # ---- tail
                maxv = scr.tile([P, T], F32, tag='maxv')
                nc.vector.tensor_copy(maxv[:], max8[:, :, 0])
                valid = scr.tile([P, T], F32, tag='valid')
                nc.vector.tensor_single_scalar(valid[:], maxv[:], -THR2,
                                               op=ALU.is_gt)

                # window-local argmin -> global sorted index (int16), then
                # rewrap for dma_gather: n-ordered via DRAM, reload (16,128)
                # wrapped, replicated into all 8 gpsimd core groups.
                idxf = scr.tile([P, T], F32, tag='idxf')
                nc.vector.tensor_copy(idxf[:], idx8[:, :, 0])
                idxg = scr.tile([P, T], F32, tag='idxg')
                nc.vector.tensor_tensor(idxg[:], idxf[:], sbase_s[:],
                                        op=ALU.add)
                idx16 = scr.tile([P, T], I16, tag='idx16')
                nc.vector.tensor_copy(idx16[:], idxg[:])
                with tc.tile_pool(name='dram', bufs=1, space='DRAM') as dp:
                    iscr = dp.tile([N], I16, tag='iscr')
                    nc.sync.dma_start(
                        iscr[:].rearrange("(t p) -> p t", p=P), idx16[:])
                    idxw = scr.tile([P, T * 8], I16, tag='idxw')
                    wrapped = iscr[:].rearrange("(s r) -> r s", r=16)
                    for g in range(8):
                        nc.sync.dma_start(idxw[16 * g:16 * (g + 1), :],
                                          wrapped)
                    sel = scr.tile([P, T, CTW], F32, tag='sel')
                    nc.gpsimd.dma_gather(
                        out_ap=sel[:], in_ap=ctab_e, idxs_ap=idxw[:],
                        num_idxs=N, num_idxs_reg=N, elem_size=CTW,
                        single_packet=False)
                dbsel = sel[:, :, 0:D]
                kbsel = sel[:, :, D:D + 4]

                prod = scr.tile([P, T, D], F32, tag='prod')
                nc.vector.tensor_tensor(prod[:], dar_s[:], dbsel,
                                        op=ALU.mult)
                simsel = scr.tile([P, T], F32, tag='simsel')
                nc.vector.tensor_reduce(simsel[:], prod[:], axis=AX.X,
                                        op=ALU.add)

                kdiff = scr.tile([P, T, 4], F32, tag='kdiff')
                nc.vector.tensor_sub(kdiff[:], kat_s[:], kbsel)
                kd2 = scr.tile([P, T, 2, 2], F32, tag='kd2')
                nc.vector.tensor_tensor(
                    kd2[:], kdiff[:].rearrange("p t (a b) -> p t a b", b=2),
                    kdiff[:].rearrange("p t (a b) -> p t a b", b=2),
                    op=ALU.mult)
                dsum = scr.tile([P, T, 2], F32, tag='dsum')
                nc.vector.tensor_reduce(dsum[:], kd2[:], axis=AX.X, op=ALU.add)
                # sqrt(x + 1e-12) = exp(0.5 * ln(x + 1e-12)); keeps ACT on
                # the {exp, ln} table set (sqrt lives in a different set).
                lnv = scr.tile([P, T, 2], F32, tag='lnv')
                _i = nc.scalar.activation(lnv[:], dsum[:], AF.Ln,
                                          bias=b_e12[:]).ins
                if last_exp[0] is not None:
                    add_dep_helper(_i, last_exp[0], False,
                                   "tail ACT after main exps")
                lns1 = scr.tile([P, T], F32, tag='lns1')
                _i = nc.scalar.activation(lns1[:], s1buf[:], AF.Ln).ins
                if last_exp[0] is not None:
                    add_dep_helper(_i, last_exp[0], False,
                                   "tail ACT after main exps")
                errs = scr.tile([P, T, 2], F32, tag='errs')
                _errs_i = nc.scalar.activation(errs[:], lnv[:], AF.Exp,
                                               scale=0.5).ins
                add_dep_helper(_errs_i, _i, False, "group Lns before Exps")
                rp = scr.tile([P, T], F32, tag='rp')
                nc.vector.tensor_reduce(rp[:], errs[:], axis=AX.X, op=ALU.add)
                rpv = scr.tile([P, T], F32, tag='rpv')
                nc.vector.tensor_tensor(rpv[:], rp[:], valid[:], op=ALU.mult)
                nc.vector.tensor_reduce(partials[:, 1:2], rpv[:], axis=AX.X,
                                        op=ALU.add)

                zsel = scr.tile([P, T], F32, tag='zsel')
                nc.vector.tensor_scalar(zsel[:], simsel[:], 10.0, -10.0,
                                        op0=ALU.mult, op1=ALU.add)
                lq = scr.tile([P, T], F32, tag='lq')
                nc.vector.tensor_sub(lq[:], zsel[:], lns1[:])
                lqv = scr.tile([P, T], F32, tag='lqv')
                nc.vector.tensor_tensor(lqv[:], lq[:], valid[:], op=ALU.mult)
                nc.vector.tensor_reduce(partials[:, 2:3], lqv[:], axis=AX.X,
                                        op=ALU.add)

                rs2 = scr.tile([P, T], F32, tag='rs2')
                nc.vector.reciprocal(rs2[:], s2buf[:])
                e1s = scr.tile([P, T], F32, tag='e1s')
                _i = nc.scalar.activation(e1s[:], simsel[:], AF.Exp).ins
                if last_exp[0] is not None:
                    add_dep_helper(_i, last_exp[0], False,
                                   "tail ACT after main exps")
                r = scr.tile([P, T], F32, tag='r')
                nc.vector.tensor_tensor(r[:], e1s[:], rs2[:], op=ALU.mult)
                sv = scr.tile([P, T], F32, tag='sv')
                nc.vector.tensor_tensor(sv[:], sa_s[:], valid[:], op=ALU.mult)
                nc.vector.tensor_reduce(partials[:, 4:5], sv[:], axis=AX.X,
                                        op=ALU.add)
                rsv = scr.tile([P, T], F32, tag='rsv')
                nc.vector.tensor_tensor(rsv[:], r[:], sv[:], op=ALU.mult)
                nc.vector.tensor_reduce(partials[:, 3:4], rsv[:], axis=AX.X,
                                        op=ALU.add)
                nc.vector.tensor_reduce(partials[:, 0:1], valid[:], axis=AX.X,
                                        op=ALU.add)

                with tc.tile_pool(name='psum_fin', bufs=1,
                                  space='PSUM') as finp:
                    pfin = finp.tile([16, 1], F32, tag='pfin')
                    nc.tensor.matmul(pfin[:], partials[:], ones_s[:],
                                     start=True, stop=True)
                    pfs = scr.tile([16, 1], F32, tag='pfs')
                    nc.vector.tensor_copy(pfs[:], pfin[:])
                    nc.sync.dma_start(out_e, pfs[:])

            for i in range(loop_n):
                body(i)

    nc.compile()
    return nc


# ------------------------------------------------------------- host combine
def _combine(partials):
    p = np.asarray(partials, np.float64).sum(axis=0)
    cnt, rp_sum, lq_sum, rsv_sum, sv_sum = p[0], p[1], p[2], p[3], p[4]
    dA, cA, dB, cB = p[5], p[6], p[7], p[8]
    loss_rp = 0.5 * rp_sum / max(cnt, 1.0) if cnt > 0 else 0.0
    loss_ds = -lq_sum / max(cnt, 1.0) if cnt > 0 else 0.0
    loss_re = (sv_sum - rsv_sum) / max(sv_sum, EPS) if sv_sum > 0 else 0.0
    da = dA / max(cA, 1.0) if cA > 0 else 0.0
    db = dB / max(cB, 1.0) if cB > 0 else 0.0
    loss_pk = 0.5 * (da + db)
    total = 1.0 * loss_rp + 0.5 * loss_pk + 5.0 * loss_ds + 1.0 * loss_re
    return np.array([loss_rp, loss_pk, loss_ds, loss_re, total], np.float32)


_NC_CACHE = {}


def _get_nc(loop_n=1, W=256):
    key = (loop_n, W)
    if key not in _NC_CACHE:
        _NC_CACHE[key] = build(loop_n, W=W)
    return _NC_CACHE[key]


def kernel(**inputs) -> np.ndarray:
    inputs = {k: np.asarray(v) for k, v in inputs.items()}
    try:
        maps = _prep(**inputs, W=256)
        nc = _get_nc(1, 256)
    except OverflowError:
        maps = _prep(**inputs, W=512)
        nc = _get_nc(1, 512)
    res = run_bass_kernel_spmd(nc, maps, core_ids=list(range(8)))
    partials = np.stack([res.results[c]['out'].reshape(16) for c in range(8)])
    return _combine(partials)


if __name__ == "__main__":
    import jax
    with jax.default_device(jax.devices('cpu')[0]):
        import reference
        inputs = {k: np.asarray(v) for k, v in reference.setup_inputs().items()}
        expected = np.asarray(reference.reference(**inputs))
    actual = kernel(**inputs)
    print("expected:", expected)
    print("actual  :", actual)
    rel = np.abs(actual - expected) / np.maximum(np.abs(expected), 1e-8)
    print("rel err :", rel, "max:", rel.max())


# revision 28
# speedup vs baseline: 16.3973x; 1.0479x over previous
# ALIKED loss wrapper — Trainium2 Bass kernel, 8-core data parallel.
#
# Sharding: pure data parallel. B=8 images, one image per NeuronCore. Every
# loss term is batch-local; the final scalar reductions (match counts / loss
# sums) come back as 16 per-core partial sums combined on the host
# (equivalent to the all-reduce in the hint, but off the critical path).
#
# Per-core device work (N=2048 keypoints, D=128, 768x768 score maps):
#   - banded NN matching: host sorts A and B keypoints by y; each 128-row
#     A-tile only needs the B-points within its y-range +-5px (the match
#     threshold), a W=256 window of the sorted order (~155 expected, +8
#     sigma margin; host raises and falls back to W=512 if ever exceeded).
#     d2neg = -(pairwise sq dist) via a K=4 matmul trick:
#     lhsT=[xw, yw, xw^2+yw^2, 1], rhs=[2xb, 2yb, -1, -(xb^2+yb^2)];
#     argmin via DVE max/max_index over the W-window.
#   - sim = desc_a @ desc_b^T (K=128 matmul, full 2048x2048); both softmax
#     denominators via ACT exp passes with fused per-row accumulation
#     (sim <= 1 so no max-subtraction is needed); ACT is the bottleneck
#     engine (2 x 2048 elem/row transcendental passes).
#   - matched-row values (desc_b, kpts_b, warped kpts_b) via one gpsimd
#     dma_gather from a combined 768B-row table using the argmin indices.
#   - dispersity (peaky) loss on host-pregathered 5x5 patches (pure O(N*25)
#     data movement; exp/softmax/dot/mean all on device).
# Host: O(N) prep (warps, sorting, floors/clips, layout), final combine.

import numpy as np

import concourse.bacc as bacc
import concourse.mybir as mybir
import concourse.tile as tile
from concourse.tile_rust import add_dep_helper
from concourse.bass_utils import run_bass_kernel_spmd

F32 = mybir.dt.float32
F32R = mybir.dt.float32r
I16 = mybir.dt.int16
U32 = mybir.dt.uint32
AF = mybir.ActivationFunctionType
ALU = mybir.AluOpType
AX = mybir.AxisListType

B, N, D, IMG = 8, 2048, 128, 768
P, T = 128, 16          # partition dim x row tiles (P*T == N)
NCH = 4                 # 512-column chunks per sim row tile
CH = N // NCH
THR = 5.0
THR2 = THR * THR        # match threshold squared (5 px)
HALF = 2
EPS = 1e-8
CTW = 192               # combined gather table row (768B, 256B multiple)


# ----------------------------------------------------------------- host prep
def _prep(kpts_a, desc_a, scores_a, score_map_a, kpts_b, desc_b, scores_b,
          score_map_b, H_ab, W=256):
    """O(N) host prep. Returns per-core input maps."""
    f32 = np.float32
    kpts_a = np.asarray(kpts_a, f32)
    kpts_b = np.asarray(kpts_b, f32)

    def warp(k, H):
        ph = np.concatenate([k, np.ones_like(k[..., :1])], axis=-1)
        pw = np.einsum('bij,bnj->bni', H.astype(f32), ph).astype(f32)
        return (pw[..., :2] / (pw[..., 2:3] + EPS)).astype(f32)

    ka_w = warp(kpts_a, np.asarray(H_ab, f32))
    H_inv = np.linalg.inv(np.asarray(H_ab, np.float64)).astype(f32)
    kb_w = warp(kpts_b, H_inv)

    def patch_prep(k):
        x = np.floor(k[..., 0]).astype(np.int32)
        y = np.floor(k[..., 1]).astype(np.int32)
        v = ((x >= HALF) & (x < IMG - HALF) & (y >= HALF) & (y < IMG - HALF))
        xc = np.clip(x, HALF, IMG - HALF - 1)
        yc = np.clip(y, HALF, IMG - HALF - 1)
        dys = np.arange(-HALF, HALF + 1, dtype=np.int32)
        pidx = (yc[..., None] + dys) * IMG + (xc[..., None] - HALF)  # (B,N,5)
        return pidx, v.astype(f32)

    pia, pva = patch_prep(kpts_a)
    pib, pvb = patch_prep(kpts_b)

    def gather_patches(score_map, pidx):
        mp = np.asarray(score_map, f32).reshape(B, IMG * IMG)
        cols = pidx[..., None] + np.arange(5, dtype=np.int32)
        return np.take_along_axis(mp, cols.reshape(B, N * 25), axis=-1) \
                 .reshape(B, N, 25)

    pat_a = gather_patches(score_map_a, pia)
    pat_b = gather_patches(score_map_b, pib)

    off = np.arange(-HALF, HALF + 1, dtype=f32)
    gy, gx = np.meshgrid(off, off, indexing='ij')
    dist = np.sqrt(gx ** 2 + gy ** 2).reshape(-1).astype(f32)
    dist25 = np.ascontiguousarray(np.broadcast_to(dist, (P, 25)))

    desc_a = np.asarray(desc_a, f32)
    desc_b = np.asarray(desc_b, f32)
    scores_a = np.asarray(scores_a, f32)

    def pt(x):  # (N, ...) -> (P, T, ...) with n = t*P + p
        return np.ascontiguousarray(
            x.reshape(T, P, *x.shape[1:]).swapaxes(0, 1))

    maps = []
    for b in range(B):
        # sort both sets by y (any permutation works — every returned
        # quantity is a permutation-invariant sum)
        pa = np.argsort(kpts_a[b, :, 1], kind='stable')
        pb = np.argsort(kpts_b[b, :, 1], kind='stable')
        kaw_s = ka_w[b][pa]; ka_s = kpts_a[b][pa]
        kb_s = kpts_b[b][pb]; kbw_s = kb_w[b][pb]
        da_s = desc_a[b][pa]; db_s = desc_b[b][pb]
        sa_s = scores_a[b][pa]
        pat_a_s = pat_a[b][pa]; pva_s = pva[b][pa]
        pat_b_s = pat_b[b][pb]; pvb_s = pvb[b][pb]

        # per-A-tile B-window starts covering [ymin-THR, ymax+THR].
        # NN matching is exact: any B-point within the 5px threshold of a
        # tile's A-point lies inside the window; outside-window points can
        # only matter for invalid (unmatched) rows where the index is
        # multiplied by valid=0 downstream.
        yb = kb_s[:, 1]
        starts = np.empty(T, np.int32)
        for t in range(T):
            ya = kaw_s[t * P:(t + 1) * P, 1]
            lo = np.searchsorted(yb, ya.min() - THR, side='left')
            hi = np.searchsorted(yb, ya.max() + THR, side='right')
            if hi - lo > W:
                raise OverflowError(f"band window {hi - lo} > W={W}")
            starts[t] = min(max(lo, 0), N - W)

        pda = np.stack([kaw_s[:, 0], kaw_s[:, 1],
                        kaw_s[:, 0] ** 2 + kaw_s[:, 1] ** 2,
                        np.ones(N, f32)], axis=0).astype(f32)       # (4,N)
        pdbw = np.empty((4, T * W), f32)
        for t in range(T):
            seg = kb_s[starts[t]:starts[t] + W]
            pdbw[0, t * W:(t + 1) * W] = 2 * seg[:, 0]
            pdbw[1, t * W:(t + 1) * W] = 2 * seg[:, 1]
            pdbw[2, t * W:(t + 1) * W] = -1.0
            pdbw[3, t * W:(t + 1) * W] = -(seg[:, 0] ** 2 + seg[:, 1] ** 2)

        ctab = np.zeros((N, CTW), f32)
        ctab[:, :D] = db_s
        ctab[:, D:D + 2] = kb_s
        ctab[:, D + 2:D + 4] = kbw_s

        maps.append(dict(
            pda=pda, pdbw=pdbw,
            dta=np.ascontiguousarray(da_s.T),
            dtb=np.ascontiguousarray(db_s.T),
            dar=np.ascontiguousarray(da_s),
            ctab=ctab,
            kat=np.ascontiguousarray(
                pt(np.concatenate([kaw_s, ka_s], axis=1)).reshape(P, T * 4)),
            sa=np.ascontiguousarray(pt(sa_s)),
            sbase=np.ascontiguousarray(
                np.broadcast_to(starts.astype(f32), (P, T))),
            pat_a=np.ascontiguousarray(pt(pat_a_s).reshape(P, T * 25)),
            pat_b=np.ascontiguousarray(pt(pat_b_s).reshape(P, T * 25)),
            pva=np.ascontiguousarray(pt(pva_s)),
            pvb=np.ascontiguousarray(pt(pvb_s)),
            dst=dist25,
        ))
    return maps


# ------------------------------------------------------------- device kernel
def build(loop_n: int = 1, W: int = 256, skip=()):
    """Build + compile the per-core Bass program (identical on all 8 cores).

    loop_n > 1 repeats the whole body (python-unrolled) for benchmarking —
    the result is rewritten identically each iteration."""
    nc = bacc.Bacc("TRN2", target_bir_lowering=False, debug=False,
                   num_devices=8)

    def din(name, shape, dt=F32):
        return nc.dram_tensor(name, shape, dt, kind="ExternalInput").ap()

    pda_e = din('pda', [4, N])
    pdbw_e = din('pdbw', [4, T * W])
    dta_e = din('dta', [D, N], F32R)
    dtb_e = din('dtb', [D, N], F32R)
    dar_e = din('dar', [N, D])
    ctab_e = din('ctab', [N, CTW])
    kat_e = din('kat', [P, T * 4])
    sa_e = din('sa', [P, T])
    sbase_e = din('sbase', [P, T])
    pat_a_e = din('pat_a', [P, T * 25])
    pat_b_e = din('pat_b', [P, T * 25])
    pva_e = din('pva', [P, T])
    pvb_e = din('pvb', [P, T])
    dst_e = din('dst', [P, 25])
    out_e = nc.dram_tensor('out', [1, 16], F32, kind="ExternalOutput").ap()

    with tile.TileContext(nc) as tc:
        with (
            tc.tile_pool(name='const', bufs=1) as cp,
            tc.tile_pool(name='work', bufs=2) as wp,
            tc.tile_pool(name='scr', bufs=1) as scr,
        ):
            def body(_i=0):
                # ---- resident loads
                pda_s = cp.tile([4, N], F32, tag='pda_s')
                nc.sync.dma_start(pda_s[:], pda_e)
                pdbw_s = cp.tile([4, T * W], F32, tag='pdbw_s')
                nc.sync.dma_start(pdbw_s[:], pdbw_e)
                # chunked loads: the first sim matmuls only dep on their own
                # slice instead of the whole 1MB tensor
                dta_s = cp.tile([D, N], F32R, tag='dta_s')
                dtb_s = cp.tile([D, N], F32R, tag='dtb_s')
                nc.sync.dma_start(dtb_s[:, :CH], dtb_e[:, :CH])
                nc.sync.dma_start(dta_s[:, :P], dta_e[:, :P])
                nc.sync.dma_start(dtb_s[:, CH:N // 2], dtb_e[:, CH:N // 2])
                nc.sync.dma_start(dtb_s[:, N // 2:], dtb_e[:, N // 2:])
                nc.sync.dma_start(dta_s[:, P:N // 2], dta_e[:, P:N // 2])
                nc.sync.dma_start(dta_s[:, N // 2:], dta_e[:, N // 2:])
                sa_s = cp.tile([P, T], F32, tag='sa_s')
                nc.sync.dma_start(sa_s[:], sa_e)
                sbase_s = cp.tile([P, T], F32, tag='sbase_s')
                nc.sync.dma_start(sbase_s[:], sbase_e)
                pva_s = cp.tile([P, T], F32, tag='pva_s')
                nc.sync.dma_start(pva_s[:], pva_e)
                pvb_s = cp.tile([P, T], F32, tag='pvb_s')
                nc.sync.dma_start(pvb_s[:], pvb_e)
                dst_s = cp.tile([P, 25], F32, tag='dst_s')
                nc.sync.dma_start(dst_s[:], dst_e)
                # tail-only loads (lower priority: emitted after the above)
                dar_s = cp.tile([P, T, D], F32, tag='dar_s')
                nc.sync.dma_start(
                    dar_s[:], dar_e.rearrange("(t p) d -> p t d", p=P))
                kat_s = cp.tile([P, T, 4], F32, tag='kat_s')
                nc.sync.dma_start(
                    kat_s[:], kat_e.rearrange("p (t c) -> p t c", c=4))

                partials = cp.tile([P, 16], F32, tag='partials')
                nc.vector.memset(partials[:], 0.0)
                ones_s = cp.tile([P, 1], F32, tag='ones_s')
                nc.vector.memset(ones_s[:], 1.0)
                b_m10 = cp.tile([P, 1], F32, tag='b_m10')
                nc.vector.memset(b_m10[:], -10.0)
                b_e12 = cp.tile([P, 1], F32, tag='b_e12')
                nc.vector.memset(b_e12[:], 1e-12)

                s1buf = cp.tile([P, T], F32, tag='s1buf')
                s2buf = cp.tile([P, T], F32, tag='s2buf')
                max8 = cp.tile([P, T, 8], F32, tag='max8')
                idx8 = cp.tile([P, T, 8], U32, tag='idx8')

                # ---- dispersity (host-gathered patches; math on device)
                for pat_e, pv_s, col in ((pat_a_e, pva_s, 5),
                                         (pat_b_e, pvb_s, 7)):
                    patches = wp.tile([P, T * 25], F32, tag='patches')
                    nc.sync.dma_start(patches[:], pat_e)
                    pex = wp.tile([P, T * 25], F32, tag='pex')
                    nc.scalar.activation(pex[:], patches[:], AF.Exp)
                    pxd = wp.tile([P, T, 25], F32, tag='pxd')
                    nc.vector.tensor_tensor(
                        pxd[:], pex[:].rearrange("p (t k) -> p t k", k=25),
                        dst_s[:].unsqueeze(1).to_broadcast([P, T, 25]),
                        op=ALU.mult)
                    numer = wp.tile([P, T], F32, tag='numer')
                    nc.vector.tensor_reduce(
                        numer[:], pxd[:], axis=AX.X, op=ALU.add)
                    denom = wp.tile([P, T], F32, tag='denom')
                    nc.vector.tensor_reduce(
                        denom[:], pex[:].rearrange("p (t k) -> p t k", k=25),
                        axis=AX.X, op=ALU.add)
                    rden = wp.tile([P, T], F32, tag='rden')
                    nc.vector.reciprocal(rden[:], denom[:])
                    disp = wp.tile([P, T], F32, tag='disp')
                    nc.vector.tensor_tensor(disp[:], numer[:], rden[:],
                                            op=ALU.mult)
                    dv = wp.tile([P, T], F32, tag='dv')
                    nc.vector.tensor_tensor(dv[:], disp[:], pv_s[:],
                                            op=ALU.mult)
                    nc.vector.tensor_reduce(partials[:, col:col + 1], dv[:],
                                            axis=AX.X, op=ALU.add)
                    nc.vector.tensor_reduce(partials[:, col + 1:col + 2],
                                            pv_s[:], axis=AX.X, op=ALU.add)

                # ---- phase 1: d2 matmuls + argmin scans (DVE-bound,
                # ~13us) with d2 on 2 PSUM banks; the first SPLIT sim tiles
                # run concurrently in a single-buffered 4-bank pool so ACT
                # starts its exp passes immediately. Phase 2: remaining sim
                # tiles double-buffered across the full 8 banks. The tail's
                # gather/DVE work overlaps phase 2.
                last_exp = [None]
                SPLIT = 4

                def sim_tile(t, pool):
                    sim_ps = pool.tile([P, N], F32, tag='sim_ps')
                    for c in range(NCH):
                        nc.tensor.matmul(
                            sim_ps[:, c * CH:(c + 1) * CH],
                            dta_s[:, t * P:(t + 1) * P],
                            dtb_s[:, c * CH:(c + 1) * CH],
                            start=True, stop=True)
                    junk = scr.tile([P, N], F32, tag='junk')
                    if 'exps' not in skip and 'accum' not in skip:
                        nc.scalar.activation(
                            junk[:], sim_ps[:], AF.Exp,
                            bias=b_m10[:], scale=10.0,
                            accum_out=s1buf[:, t:t + 1])
                        last_exp[0] = nc.scalar.activation(
                            junk[:], sim_ps[:], AF.Exp,
                            accum_out=s2buf[:, t:t + 1]).ins
                    elif 'exps' not in skip:
                        # timing ablation: exps without fused accumulation
                        nc.scalar.activation(junk[:], sim_ps[:], AF.Exp,
                                             bias=b_m10[:], scale=10.0)
                        last_exp[0] = nc.scalar.activation(
                            junk[:], sim_ps[:], AF.Exp).ins
                        nc.vector.memset(s1buf[:, t:t + 1], 1.0)
                        nc.vector.memset(s2buf[:, t:t + 1], 1.0)
                    else:
                        nc.vector.memset(s1buf[:, t:t + 1], 1.0)
                        nc.vector.memset(s2buf[:, t:t + 1], 1.0)

                with (
                    tc.tile_pool(name='psum_d2', bufs=2, space='PSUM')
                        as d2p,
                    tc.tile_pool(name='psum_sim1', bufs=1, space='PSUM')
                        as simp1,
                ):
                    for t in range(SPLIT):
                        sim_tile(t, simp1)
                    for t in range(T):
                        d2ps = d2p.tile([P, W], F32, tag='d2ps')
                        nc.tensor.matmul(
                            d2ps[:],
                            pda_s[:, t * P:(t + 1) * P],
                            pdbw_s[:, t * W:(t + 1) * W],
                            start=True, stop=True)
                        if 'scan' not in skip:
                            nc.vector.max(max8[:, t, :], d2ps[:])
                            nc.vector.max_index(idx8[:, t, :],
                                                max8[:, t, :], d2ps[:])
                        else:
                            nc.vector.memset(max8[:, t, :], 0.0)
                            nc.vector.memset(idx8[:, t, :], 0)

                with tc.tile_pool(name='psum_sim', bufs=2,
                                  space='PSUM') as simp:
                    for t in range(SPLIT, T):
                        sim_tile(t, simp)

                # ---- tail
                maxv = scr.tile([P, T], F32, tag='maxv')
                nc.vector.tensor_copy(maxv[:], max8[:, :, 0])
                valid = scr.tile([P, T], F32, tag='valid')
                nc.vector.tensor_single_scalar(valid[:], maxv[:], -THR2,
                                               op=ALU.is_gt)

                # window-local argmin -> global sorted index (int16), then
                # rewrap for dma_gather: n-ordered via DRAM, reload (16,128)
                # wrapped, replicated into all 8 gpsimd core groups.
                idxf = scr.tile([P, T], F32, tag='idxf')
                nc.vector.tensor_copy(idxf[:], idx8[:, :, 0])
                idxg = scr.tile([P, T], F32, tag='idxg')
                nc.vector.tensor_tensor(idxg[:], idxf[:], sbase_s[:],
                                        op=ALU.add)
                idx16 = scr.tile([P, T], I16, tag='idx16')
                nc.vector.tensor_copy(idx16[:], idxg[:])
                with tc.tile_pool(name='dram', bufs=1, space='DRAM') as dp:
                    iscr = dp.tile([N], I16, tag='iscr')
                    nc.sync.dma_start(
                        iscr[:].rearrange("(t p) -> p t", p=P), idx16[:])
                    idxw = scr.tile([P, T * 8], I16, tag='idxw')
                    wrapped = iscr[:].rearrange("(s r) -> r s", r=16)
                    for g in range(8):
                        nc.sync.dma_start(idxw[16 * g:16 * (g + 1), :],
                                          wrapped)
                    sel = scr.tile([P, T, CTW], F32, tag='sel')
                    nc.gpsimd.dma_gather(
                        out_ap=sel[:], in_ap=ctab_e, idxs_ap=idxw[:],
                        num_idxs=N, num_idxs_reg=N, elem_size=CTW,
                        single_packet=False)
                dbsel = sel[:, :, 0:D]
                kbsel = sel[:, :, D:D + 4]

                prod = scr.tile([P, T, D], F32, tag='prod')
                nc.vector.tensor_tensor(prod[:], dar_s[:], dbsel,
                                        op=ALU.mult)
                simsel = scr.tile([P, T], F32, tag='simsel')
                nc.vector.tensor_reduce(simsel[:], prod[:], axis=AX.X,
                                        op=ALU.add)

                kdiff = scr.tile([P, T, 4], F32, tag='kdiff')
                nc.vector.tensor_sub(kdiff[:], kat_s[:], kbsel)
                kd2 = scr.tile([P, T, 2, 2], F32, tag='kd2')
                nc.vector.tensor_tensor(
                    kd2[:], kdiff[:].rearrange("p t (a b) -> p t a b", b=2),
                    kdiff[:].rearrange("p t (a b) -> p t a b", b=2),
                    op=ALU.mult)
                dsum = scr.tile([P, T, 2], F32, tag='dsum')
                nc.vector.tensor_reduce(dsum[:], kd2[:], axis=AX.X, op=ALU.add)
                # sqrt(x + 1e-12) = exp(0.5 * ln(x + 1e-12)); keeps ACT on
                # the {exp, ln} table set (sqrt lives in a different set).
                lnv = scr.tile([P, T, 2], F32, tag='lnv')
                _i = nc.scalar.activation(lnv[:], dsum[:], AF.Ln,
                                          bias=b_e12[:]).ins
                if last_exp[0] is not None:
                    add_dep_helper(_i, last_exp[0], False,
                                   "tail ACT after main exps")
                lns1 = scr.tile([P, T], F32, tag='lns1')
                _i = nc.scalar.activation(lns1[:], s1buf[:], AF.Ln).ins
                if last_exp[0] is not None:
                    add_dep_helper(_i, last_exp[0], False,
                                   "tail ACT after main exps")
                errs = scr.tile([P, T, 2], F32, tag='errs')
                _errs_i = nc.scalar.activation(errs[:], lnv[:], AF.Exp,
                                               scale=0.5).ins
                add_dep_helper(_errs_i, _i, False, "group Lns before Exps")
                rp = scr.tile([P, T], F32, tag='rp')
                nc.vector.tensor_reduce(rp[:], errs[:], axis=AX.X, op=ALU.add)
                rpv = scr.tile([P, T], F32, tag='rpv')
                nc.vector.tensor_tensor(rpv[:], rp[:], valid[:], op=ALU.mult)
                nc.vector.tensor_reduce(partials[:, 1:2], rpv[:], axis=AX.X,
                                        op=ALU.add)

                zsel = scr.tile([P, T], F32, tag='zsel')
                nc.vector.tensor_scalar(zsel[:], simsel[:], 10.0, -10.0,
                                        op0=ALU.mult, op1=ALU.add)
                lq = scr.tile([P, T], F32, tag='lq')
                nc.vector.tensor_sub(lq[:], zsel[:], lns1[:])
                lqv = scr.tile([P, T], F32, tag='lqv')
                nc.vector.tensor_tensor(lqv[:], lq[:], valid[:], op=ALU.mult)
                nc.vector.tensor_reduce(partials[:, 2:3], lqv[:], axis=AX.X,
                                        op=ALU.add)

                rs2 = scr.tile([P, T], F32, tag='rs2')
                nc.vector.reciprocal(rs2[:], s2buf[:])
                e1s = scr.tile([P, T], F32, tag='e1s')
                _i = nc.scalar.activation(e1s[:], simsel[:], AF.Exp).ins
                if last_exp[0] is not None:
                    add_dep_helper(_i, last_exp[0], False,
                                   "tail ACT after main exps")
                r = scr.tile([P, T], F32, tag='r')
                nc.vector.tensor_tensor(r[:], e1s[:], rs2[:], op=ALU.mult)
                sv = scr.tile([P, T], F32, tag='sv')
                nc.vector.tensor_tensor(sv[:], sa_s[:], valid[:], op=ALU.mult)
                nc.vector.tensor_reduce(partials[:, 4:5], sv[:], axis=AX.X,
                                        op=ALU.add)
                rsv = scr.tile([P, T], F32, tag='rsv')
                nc.vector.tensor_tensor(rsv[:], r[:], sv[:], op=ALU.mult)
                nc.vector.tensor_reduce(partials[:, 3:4], rsv[:], axis=AX.X,
                                        op=ALU.add)
                nc.vector.tensor_reduce(partials[:, 0:1], valid[:], axis=AX.X,
                                        op=ALU.add)

                with tc.tile_pool(name='psum_fin', bufs=1,
                                  space='PSUM') as finp:
                    pfin = finp.tile([16, 1], F32, tag='pfin')
                    nc.tensor.matmul(pfin[:], partials[:], ones_s[:],
                                     start=True, stop=True)
                    pfs = scr.tile([16, 1], F32, tag='pfs')
                    nc.vector.tensor_copy(pfs[:], pfin[:])
                    nc.sync.dma_start(out_e, pfs[:])

            for i in range(loop_n):
                body(i)

    nc.compile()
    return nc


# ------------------------------------------------------------- host combine
def _combine(partials):
    p = np.asarray(partials, np.float64).sum(axis=0)
    cnt, rp_sum, lq_sum, rsv_sum, sv_sum = p[0], p[1], p[2], p[3], p[4]
    dA, cA, dB, cB = p[5], p[6], p[7], p[8]
    loss_rp = 0.5 * rp_sum / max(cnt, 1.0) if cnt > 0 else 0.0
    loss_ds = -lq_sum / max(cnt, 1.0) if cnt > 0 else 0.0
    loss_re = (sv_sum - rsv_sum) / max(sv_sum, EPS) if sv_sum > 0 else 0.0
    da = dA / max(cA, 1.0) if cA > 0 else 0.0
    db = dB / max(cB, 1.0) if cB > 0 else 0.0
    loss_pk = 0.5 * (da + db)
    total = 1.0 * loss_rp + 0.5 * loss_pk + 5.0 * loss_ds + 1.0 * loss_re
    return np.array([loss_rp, loss_pk, loss_ds, loss_re, total], np.float32)


_NC_CACHE = {}


def _get_nc(loop_n=1, W=256):
    key = (loop_n, W)
    if key not in _NC_CACHE:
        _NC_CACHE[key] = build(loop_n, W=W)
    return _NC_CACHE[key]


def kernel(**inputs) -> np.ndarray:
    inputs = {k: np.asarray(v) for k, v in inputs.items()}
    try:
        maps = _prep(**inputs, W=256)
        nc = _get_nc(1, 256)
    except OverflowError:
        maps = _prep(**inputs, W=512)
        nc = _get_nc(1, 512)
    res = run_bass_kernel_spmd(nc, maps, core_ids=list(range(8)))
    partials = np.stack([res.results[c]['out'].reshape(16) for c in range(8)])
    return _combine(partials)


if __name__ == "__main__":
    import jax
    with jax.default_device(jax.devices('cpu')[0]):
        import reference
        inputs = {k: np.asarray(v) for k, v in reference.setup_inputs().items()}
        expected = np.asarray(reference.reference(**inputs))
    actual = kernel(**inputs)
    print("expected:", expected)
    print("actual  :", actual)
    rel = np.abs(actual - expected) / np.maximum(np.abs(expected), 1e-8)
    print("rel err :", rel, "max:", rel.max())
